# revision 1
# baseline (speedup 1.0000x reference)
"""Trainium2 Bass kernel: Ernie4.5 VisionAttention (varlen attention, 4x512
segments, 16 heads x 80 dim, embed 1280).

Sharding: 8 cores = 2 segment-groups (2x512 tokens each) x 4 head-groups
(4 heads each). Tensor-parallel over heads (qkv column-shard, proj row-shard),
data-parallel over segment pairs. No collectives: per-core proj partials are
summed on the host.

Compute dtype: bf16 operands, fp32 PSUM accumulation.
"""

import sys

if "/opt/trn_rl_repo" not in sys.path:
    sys.path.insert(0, "/opt/trn_rl_repo")

import numpy as np
import ml_dtypes

BF = ml_dtypes.bfloat16

EMBED = 1280
HEADS = 16
HD = 80          # head dim
RH = 40          # rotary half
SEQ = 2048
SEGLEN = 512
N_CORES = 8
HPC = 4          # heads per core
TOK = 1024       # tokens per core (2 segments)
NSEG = 2
NUNITS = 2 * HPC # q units 0..3, k units 4..7
VW = HD          # v block width per head (denominator computed separately)
VTOT = HPC * VW  # 320
SCALE = HD ** -0.5
KCH = EMBED // 128  # 10

_CACHE = {}


def _build_program():
    import concourse.tile as tile
    from concourse import bacc, mybir

    f32 = mybir.dt.float32
    bf16 = mybir.dt.bfloat16
    AF = mybir.ActivationFunctionType
    ALU = mybir.AluOpType

    nc = bacc.Bacc("TRN2", target_bir_lowering=False, debug=False,
                   num_devices=N_CORES)

    xt_d = nc.dram_tensor("xt", [EMBED + 1, TOK], bf16, kind="ExternalInput").ap()
    wqk_d = nc.dram_tensor("wqk", [EMBED, NUNITS * HD], bf16, kind="ExternalInput").ap()
    wv_d = nc.dram_tensor("wv", [EMBED + 1, VTOT], bf16, kind="ExternalInput").ap()
    wp_d = nc.dram_tensor("wp", [HPC * HD, EMBED], bf16, kind="ExternalInput").ap()
    bias_d = nc.dram_tensor("biasqk", [128, 5], f32, kind="ExternalInput").ap()
    cos_d = nc.dram_tensor("cosm", [NUNITS * HD, TOK], bf16, kind="ExternalInput").ap()
    sin_d = nc.dram_tensor("sinm", [NUNITS * HD, TOK], bf16, kind="ExternalInput").ap()
    pit_d = nc.dram_tensor("pit", [NUNITS * HD, NUNITS * HD], bf16, kind="ExternalInput").ap()
    out_d = nc.dram_tensor("outT", [EMBED, TOK], f32, kind="ExternalOutput").ap()

    with tile.TileContext(nc) as tc:
        with tc.tile_pool(name="persist", bufs=1) as P:
            # ---- persistent SBUF loads ----
            # interleave xt/wqk chunk loads so the first qk matmuls can start
            # after the first chunks land instead of after all input DMAs
            bias_sb = P.tile([128, 5], f32, name="biasqk_sb", tag="biasqk")
            nc.sync.dma_start(bias_sb[:], bias_d[:])
            xt_sb = []
            wqk_sb = []
            wv_sb = []
            for e in range(KCH):
                t = P.tile([128, TOK], bf16, name=f"xt{e}", tag=f"xt{e}")
                nc.sync.dma_start(t[:], xt_d[128 * e:128 * (e + 1), :])
                xt_sb.append(t)
                t = P.tile([128, NUNITS * HD], bf16, name=f"wqk{e}", tag=f"wqk{e}")
                nc.sync.dma_start(t[:], wqk_d[128 * e:128 * (e + 1), :])
                wqk_sb.append(t)

            ones_sb = P.tile([128, 1], bf16, name="ones_sb", tag="ones")
            nc.gpsimd.memset(ones_sb[:], 1.0)


            for e in range(KCH):
                t = P.tile([128, VTOT], bf16, name=f"wv{e}", tag=f"wv{e}")
                nc.sync.dma_start(t[:], wv_d[128 * e:128 * (e + 1), :])
                wv_sb.append(t)
            wvlast = P.tile([1, VTOT], bf16, name="wvlast", tag="wvlast")
            nc.sync.dma_start(wvlast[:], wv_d[EMBED:EMBED + 1, :])
            xlast = P.tile([1, TOK], bf16, name="xlast", tag="xlast")
            nc.sync.dma_start(xlast[:], xt_d[EMBED:EMBED + 1, :])
            cos_sb = []
            sin_sb = []
            pit_sb = []
            for t in range(5):
                c = P.tile([128, TOK], bf16, name=f"cosp{t}", tag=f"cosp{t}")
                nc.sync.dma_start(c[:], cos_d[128 * t:128 * (t + 1), :])
                cos_sb.append(c)
                sn = P.tile([128, TOK], bf16, name=f"sinp{t}", tag=f"sinp{t}")
                nc.sync.dma_start(sn[:], sin_d[128 * t:128 * (t + 1), :])
                sin_sb.append(sn)
                pt = P.tile([128, NUNITS * HD], bf16, name=f"pitp{t}", tag=f"pitp{t}")
                nc.sync.dma_start(pt[:], pit_d[128 * t:128 * (t + 1), :])
                pit_sb.append(pt)

            wp_sb = []
            for j in range(HPC):
                t = P.tile([HD, EMBED], bf16, name=f"wp{j}", tag=f"wp{j}")
                nc.sync.dma_start(t[:], wp_d[HD * j:HD * (j + 1), :])
                wp_sb.append(t)

            # persistent intermediates
            qkp_sb = [[None] * NSEG for _ in range(5)]
            UNPACK_PIECES = {t: [] for t in range(5)}
            for u in range(NUNITS):
                a = HD * u
                while a < HD * (u + 1):
                    t = a // 128
                    b = min(HD * (u + 1), 128 * (t + 1))
                    UNPACK_PIECES[t].append((u, a - HD * u, t, a - 128 * t, b - a))
                    a = b
            qrot = [P.tile([HD, TOK], bf16, name=f"qrot{u}", tag=f"qrot{u}")
                    for u in range(NUNITS)]
            v_sb = [P.tile([128, VTOT], bf16, name=f"vsb{m}", tag=f"vsb{m}")
                    for m in range(TOK // 128)]
            ctxn = [[P.tile([HD, SEGLEN], bf16, name=f"ctxn{j}_{s}", tag=f"ctxn{j}_{s}")
                     for s in range(NSEG)] for j in range(HPC)]

            with tc.tile_pool(name="ps_qkv", bufs=2, space="PSUM") as PSQ, \
                 tc.tile_pool(name="ps_swap", bufs=2, space="PSUM") as PSW, \
                 tc.tile_pool(name="ps_v", bufs=2, space="PSUM") as PSV, \
                 tc.tile_pool(name="work", bufs=3) as W:
                # phase B: packed qk projection + packed rotary, then DMA
                # unpack into per-head base-0 tiles; phase C: v projection
                # Pi-swap source blocks per packed tile t (rows shift by +-40
                # inside each 80-row unit => sources span tiles t-1..t+1)
                PI_BLOCKS = {0: [0, 1], 1: [0, 1, 2], 2: [1, 2, 3],
                             3: [2, 3, 4], 4: [3, 4]}
                for t in range(5):
                    for s in range(NSEG):
                        sc = slice(SEGLEN * s, SEGLEN * (s + 1))
                        qk_ps = PSQ.tile([128, SEGLEN], f32, name=f"qkps{t}_{s}",
                                         tag="qkps")
                        for e in range(KCH):
                            nc.tensor.matmul(qk_ps[:],
                                             wqk_sb[e][:, 128 * t:128 * (t + 1)],
                                             xt_sb[e][:, sc],
                                             start=(e == 0), stop=(e == KCH - 1))
                        q_sb = W.tile([128, SEGLEN], bf16, name=f"qsb{t}_{s}",
                                      tag="qsb", bufs=10)
                        nc.scalar.activation(q_sb[:], qk_ps[:], AF.Identity,
                                             bias=bias_sb[:, t:t + 1])
                        qkp_sb[t][s] = q_sb

                    # interleave v chunks to keep PE streaming
                    for m in (t, 5 + t):
                        if m >= TOK // 128:
                            continue
                        mc = slice(128 * m, 128 * (m + 1))
                        v_ps = PSV.tile([128, VTOT], f32, name=f"vps{m}", tag="vps")
                        for e in range(KCH):
                            nc.tensor.matmul(v_ps[:], xt_sb[e][:, mc], wv_sb[e][:],
                                             start=(e == 0), stop=False)
                        nc.tensor.matmul(v_ps[:], xlast[:, mc], wvlast[:],
                                         start=False, stop=True)
                        nc.vector.tensor_copy(v_sb[m][:], v_ps[:])

                    # Pi-swap + rotary for every packed tile whose sources are
                    # now complete (sources span t-1..t+1)
                    ready = [t - 1] if t < 4 else [3, 4]
                    for tr in ready:
                        if tr < 0:
                            continue
                        for s in range(NSEG):
                            sc = slice(SEGLEN * s, SEGLEN * (s + 1))
                            qsw_ps = PSW.tile([128, SEGLEN], f32,
                                              name=f"qsw{tr}_{s}", tag="qsw")
                            srcs = PI_BLOCKS[tr]
                            for i, tp in enumerate(srcs):
                                nc.tensor.matmul(qsw_ps[:],
                                                 pit_sb[tp][:, 128 * tr:128 * (tr + 1)],
                                                 qkp_sb[tp][s][:],
                                                 start=(i == 0),
                                                 stop=(i == len(srcs) - 1))
                            t1 = W.tile([128, SEGLEN], f32, name=f"t1_{tr}_{s}",
                                        tag="t1")
                            nc.vector.tensor_tensor(t1[:], qkp_sb[tr][s][:],
                                                    cos_sb[tr][:, sc], ALU.mult)
                            t2 = W.tile([128, SEGLEN], f32, name=f"t2_{tr}_{s}",
                                        tag="t2")
                            nc.vector.tensor_tensor(t2[:], qsw_ps[:],
                                                    sin_sb[tr][:, sc], ALU.mult)
                            rp = W.tile([128, SEGLEN], bf16, name=f"rotp{tr}_{s}",
                                        tag="rotp", bufs=6)
                            nc.vector.tensor_tensor(rp[:], t1[:], t2[:], ALU.add)
                            for (u, po, tt, toff, ln) in UNPACK_PIECES[tr]:
                                nc.sync.dma_start(qrot[u][po:po + ln, sc],
                                                  rp[toff:toff + ln, :])

            # ---- phase D: attention per (segment s, head j) ----
            with tc.tile_pool(name="ps_st", bufs=4, space="PSUM") as PST, \
                 tc.tile_pool(name="ps_ctx", bufs=2, space="PSUM") as PSC, \
                 tc.tile_pool(name="ps_den", bufs=2, space="PSUM") as PSD, \
                 tc.tile_pool(name="workd", bufs=6) as WD:
                for j in range(HPC):
                    for s in range(NSEG):
                        sc = slice(SEGLEN * s, SEGLEN * (s + 1))
                        est = []
                        for tkc in range(SEGLEN // 128):
                            kc = slice(SEGLEN * s + 128 * tkc,
                                       SEGLEN * s + 128 * (tkc + 1))
                            st_ps = PST.tile([128, SEGLEN], f32,
                                             name=f"st{j}_{s}_{tkc}", tag="stps")
                            nc.tensor.matmul(st_ps[:], qrot[HPC + j][:, kc],
                                             qrot[j][:, sc], start=True, stop=True)
                            e_sb = WD.tile([128, SEGLEN], bf16,
                                           name=f"est{j}_{s}_{tkc}", tag="est",
                                           bufs=8)
                            nc.scalar.activation(e_sb[:], st_ps[:], AF.Exp)
                            est.append(e_sb)
                        ctx_ps = PSC.tile([HD, SEGLEN], f32,
                                          name=f"ctxps{j}_{s}", tag="ctxps")
                        den_ps = PSD.tile([1, SEGLEN], f32,
                                          name=f"denps{j}_{s}", tag="denps")
                        for tkc in range(SEGLEN // 128):
                            nc.tensor.matmul(ctx_ps[:],
                                             v_sb[4 * s + tkc][:, VW * j:VW * (j + 1)],
                                             est[tkc][:],
                                             start=(tkc == 0), stop=(tkc == 3))
                            nc.tensor.matmul(den_ps[:], ones_sb[:], est[tkc][:],
                                             start=(tkc == 0), stop=(tkc == 3))
                        rec = WD.tile([1, SEGLEN], f32, name=f"rec{j}_{s}", tag="rec")
                        nc.vector.reciprocal_approx_fast(rec[:], den_ps[:])
                        bc = WD.tile([HD, SEGLEN], f32, name=f"bc{j}_{s}", tag="bc")
                        nc.gpsimd.partition_broadcast(bc[:], rec[:])
                        nc.vector.tensor_tensor(ctxn[j][s][:], ctx_ps[:],
                                                bc[:], ALU.mult)

            # ---- phase E: output projection ----
            with tc.tile_pool(name="ps_o", bufs=4, space="PSUM") as PSO, \
                 tc.tile_pool(name="worke", bufs=4) as W2:
                for e in range(KCH):
                    for s in range(NSEG):
                        o_ps = PSO.tile([128, SEGLEN], f32, name=f"ops{e}_{s}",
                                        tag="ops")
                        for j in range(HPC):
                            nc.tensor.matmul(o_ps[:],
                                             wp_sb[j][:, 128 * e:128 * (e + 1)],
                                             ctxn[j][s][:],
                                             start=(j == 0), stop=(j == HPC - 1))
                        o_sb = W2.tile([128, SEGLEN], f32, name=f"osb{e}_{s}",
                                       tag="osb")
                        nc.vector.tensor_copy(o_sb[:], o_ps[:])
                        nc.sync.dma_start(
                            out_d[128 * e:128 * (e + 1),
                                  SEGLEN * s:SEGLEN * (s + 1)], o_sb[:])

    nc.compile()
    return nc


def _prep_inputs(x, rotary_pos_emb, qkv_w, qkv_b):
    """Build per-core input shards (host-side layout/constant prep)."""
    x2 = np.asarray(x, np.float32).reshape(SEQ, EMBED)
    rope = np.asarray(rotary_pos_emb, np.float32)
    qkv_w = np.asarray(qkv_w, np.float32)
    qkv_b = np.asarray(qkv_b, np.float32)

    # packed rotary multipliers: packed row p = 80u + d -> r = d % 40
    r_idx = np.tile(np.arange(HD) % RH, NUNITS)      # [640]
    cos_full = np.cos(rope)[:, r_idx].T.astype(BF)   # [640, 2048]
    sin_full = np.sin(rope)[:, r_idx].T.astype(BF)

    # packed swap permutation (sign folded), block-diagonal per 80-row unit:
    # within a unit, row d<40 reads -(d+40), row d>=40 reads +(d-40)
    D = NUNITS * HD
    Pi = np.zeros((D, D), np.float32)
    for u in range(NUNITS):
        o = HD * u
        for i in range(RH):
            Pi[o + i, o + i + RH] = -1.0
            Pi[o + i + RH, o + i] = 1.0
    pit = np.ascontiguousarray(Pi.T).astype(BF)

    in_maps = []
    for c in range(N_CORES):
        sg, hg = divmod(c, HPC)
        toks = slice(TOK * sg, TOK * (sg + 1))
        heads = [HPC * hg + j for j in range(HPC)]

        xa = np.empty((EMBED + 1, TOK), np.float32)
        xa[:EMBED] = x2[toks].T
        xa[EMBED] = 1.0

        wqk = np.empty((EMBED, NUNITS * HD), np.float32)
        bias_flat = np.empty(NUNITS * HD, np.float32)
        for j, h in enumerate(heads):
            uq, uk = j, HPC + j
            wqk[:, HD * uq:HD * (uq + 1)] = qkv_w[HD * h:HD * (h + 1), :].T * SCALE
            bias_flat[HD * uq:HD * (uq + 1)] = qkv_b[HD * h:HD * (h + 1)] * SCALE
            ko = EMBED + HD * h
            wqk[:, HD * uk:HD * (uk + 1)] = qkv_w[ko:ko + HD, :].T
            bias_flat[HD * uk:HD * (uk + 1)] = qkv_b[ko:ko + HD]
        bias = np.ascontiguousarray(bias_flat.reshape(5, 128).T)

        wv = np.zeros((EMBED + 1, VTOT), np.float32)
        for j, h in enumerate(heads):
            vo = 2 * EMBED + HD * h
            wv[:EMBED, VW * j:VW * j + HD] = qkv_w[vo:vo + HD, :].T
            wv[EMBED, VW * j:VW * j + HD] = qkv_b[vo:vo + HD]

        wp = np.empty((HPC * HD, EMBED), np.float32)
        for j, h in enumerate(heads):
            wp[HD * j:HD * (j + 1), :] = _PROJ_W[:, HD * h:HD * (h + 1)].T

        in_maps.append({
            "xt": xa.astype(BF),
            "wqk": wqk.astype(BF),
            "wv": wv.astype(BF),
            "wp": wp.astype(BF),
            "biasqk": bias,
            "cosm": np.ascontiguousarray(cos_full[:, toks]),
            "sinm": np.ascontiguousarray(sin_full[:, toks]),
            "pit": pit,
        })
    return in_maps


_PROJ_W = None


def run_on_device(inputs, trace=False, trace_cores=None):
    """Shard, run on 8 NeuronCores, gather. Returns (output, BassKernelResults)."""
    global _PROJ_W
    from concourse import bass_utils

    x = np.asarray(inputs["x"], np.float32)
    cu = np.asarray(inputs["cu_seqlens"]).tolist()
    assert cu == [0, 512, 1024, 1536, 2048], (
        f"kernel compiled for 4x512 segments, got cu_seqlens={cu}")
    assert x.shape == (SEQ, 1, EMBED)

    _PROJ_W = np.asarray(inputs["proj_w"], np.float32)
    in_maps = _prep_inputs(x, inputs["rotary_pos_emb"],
                           inputs["qkv_w"], inputs["qkv_b"])

    if "nc" not in _CACHE:
        _CACHE["nc"] = _build_program()
    nc = _CACHE["nc"]

    kw = {}
    if trace:
        kw = dict(trace=True, trace_cores=trace_cores or [0])
    res = bass_utils.run_bass_kernel_spmd(nc, in_maps,
                                          core_ids=list(range(N_CORES)), **kw)

    proj_b = np.asarray(inputs["proj_b"], np.float32)
    out = np.empty((SEQ, EMBED), np.float32)
    for sg in range(2):
        acc = res.results[HPC * sg + 0]["outT"].astype(np.float32).copy()
        for hg in range(1, HPC):
            acc += res.results[HPC * sg + hg]["outT"]
        out[TOK * sg:TOK * (sg + 1)] = acc.T
    out += proj_b
    return out.reshape(SEQ, 1, EMBED), res


def kernel(**inputs):
    out, _ = run_on_device(inputs, trace=False)
    return out



# revision 11
# speedup vs baseline: 1.0839x; 1.0839x over previous
"""Trainium2 Bass kernel: Ernie4.5 VisionAttention (varlen attention, 4x512
segments, 16 heads x 80 dim, embed 1280).

Sharding: 8 cores = 2 segment-groups (2x512 tokens each) x 4 head-groups
(4 heads each). Tensor-parallel over heads (qkv column-shard, proj row-shard),
data-parallel over segment pairs. No collectives: per-core proj partials are
summed on the host.

v2 layout: heads interleaved in the packed qk projection [q0 k0 q1 k1 ...]
so head j's rotated q/k depends only on packed tiles ~j (attention overlaps
the projection). Softmax denominator rides the V matmul as an 81st ones
column per head. Output in fp16. Rotary-unpack DMAs go on the scalar-engine
HWDGE ring to stay clear of the input-load queue.

Compute dtype: bf16 operands, fp32 PSUM accumulation.
"""

import sys

if "/opt/trn_rl_repo" not in sys.path:
    sys.path.insert(0, "/opt/trn_rl_repo")

import numpy as np
import ml_dtypes

BF = ml_dtypes.bfloat16

EMBED = 1280
HEADS = 16
HD = 80          # head dim
RH = 40          # rotary half
SEQ = 2048
SEGLEN = 512
N_CORES = 8
HPC = 4          # heads per core
TOK = 1024       # tokens per core (2 segments)
NSEG = 2
NUNITS = 2 * HPC # unit 2j = q of head j, unit 2j+1 = k of head j
VW = HD          # v block width per head
VTOT = HPC * VW  # 320
SCALE = HD ** -0.5
KCH = EMBED // 128  # 10

_CACHE = {}

# unpack pieces: packed row 80u+d lives in tile t=(80u+d)//128; piece list
# per packed tile t: (unit, unit_row_offset, tile_row_offset, length)
UNPACK_PIECES = {t: [] for t in range(5)}
for _u in range(NUNITS):
    _a = HD * _u
    while _a < HD * (_u + 1):
        _t = _a // 128
        _b = min(HD * (_u + 1), 128 * (_t + 1))
        UNPACK_PIECES[_t].append((_u, _a - HD * _u, _a - 128 * _t, _b - _a))
        _a = _b

# pi-swap source blocks per packed tile t (rows shift by +-40 inside each
# 80-row unit => sources span tiles t-1..t+1)
PI_BLOCKS = {0: [0, 1], 1: [0, 1, 2], 2: [1, 2, 3], 3: [2, 3, 4], 4: [3, 4]}


def _build_program():
    import concourse.tile as tile
    from concourse import bacc, mybir

    f32 = mybir.dt.float32
    f16 = mybir.dt.float16
    bf16 = mybir.dt.bfloat16
    AF = mybir.ActivationFunctionType
    ALU = mybir.AluOpType

    nc = bacc.Bacc("TRN2", target_bir_lowering=False, debug=False,
                   num_devices=N_CORES)

    xt_d = nc.dram_tensor("xt", [EMBED, TOK], bf16, kind="ExternalInput").ap()
    wqk_d = nc.dram_tensor("wqk", [EMBED, NUNITS * HD], bf16,
                           kind="ExternalInput").ap()
    wv_d = nc.dram_tensor("wv", [EMBED, VTOT], bf16, kind="ExternalInput").ap()
    vpat_d = nc.dram_tensor("vpat", [128, VTOT], bf16,
                            kind="ExternalInput").ap()
    wp_d = nc.dram_tensor("wp", [HPC * HD, EMBED], bf16,
                          kind="ExternalInput").ap()
    bias_d = nc.dram_tensor("biasqk", [128, 5], f32, kind="ExternalInput").ap()
    cos_d = nc.dram_tensor("cosm", [NUNITS * HD, TOK], bf16,
                           kind="ExternalInput").ap()
    sin_d = nc.dram_tensor("sinm", [NUNITS * HD, TOK], bf16,
                           kind="ExternalInput").ap()
    pit_d = nc.dram_tensor("pit", [NUNITS * HD, NUNITS * HD], bf16,
                           kind="ExternalInput").ap()
    out_d = nc.dram_tensor("outT", [EMBED, TOK], f16, kind="ExternalOutput").ap()

    with tile.TileContext(nc) as tc:
        with tc.tile_pool(name="persist", bufs=1) as P:
            # ---- persistent SBUF loads, in consumption order ----
            bias_sb = P.tile([128, 5], f32, name="biasqk_sb", tag="biasqk")
            nc.sync.dma_start(bias_sb[:], bias_d[:])
            xt_sb = []
            wqk_sb = []
            wv_sb = []
            for e in range(KCH):
                t = P.tile([128, TOK], bf16, name=f"xt{e}", tag=f"xt{e}")
                nc.sync.dma_start(t[:], xt_d[128 * e:128 * (e + 1), :])
                xt_sb.append(t)
                t = P.tile([128, NUNITS * HD], bf16, name=f"wqk{e}",
                           tag=f"wqk{e}")
                nc.sync.dma_start(t[:], wqk_d[128 * e:128 * (e + 1), :])
                wqk_sb.append(t)
                t = P.tile([128, VTOT], bf16, name=f"wv{e}", tag=f"wv{e}")
                nc.sync.dma_start(t[:], wv_d[128 * e:128 * (e + 1), :])
                wv_sb.append(t)
            vpat_sb = P.tile([128, VTOT], bf16, name="vpat_sb", tag="vpat")
            nc.sync.dma_start(vpat_sb[:], vpat_d[:])
            cos_sb = []
            sin_sb = []
            pit_sb = []
            for t in range(5):
                c = P.tile([128, TOK], bf16, name=f"cosp{t}", tag=f"cosp{t}")
                nc.sync.dma_start(c[:], cos_d[128 * t:128 * (t + 1), :])
                cos_sb.append(c)
                sn = P.tile([128, TOK], bf16, name=f"sinp{t}", tag=f"sinp{t}")
                nc.sync.dma_start(sn[:], sin_d[128 * t:128 * (t + 1), :])
                sin_sb.append(sn)
                pt = P.tile([128, NUNITS * HD], bf16, name=f"pitp{t}",
                            tag=f"pitp{t}")
                nc.sync.dma_start(pt[:], pit_d[128 * t:128 * (t + 1), :])
                pit_sb.append(pt)
            wp_sb = []
            for j in range(HPC):
                t = P.tile([HD, EMBED], bf16, name=f"wp{j}", tag=f"wp{j}")
                nc.sync.dma_start(t[:], wp_d[HD * j:HD * (j + 1), :])
                wp_sb.append(t)

            # persistent intermediates
            qkp_sb = [[None] * NSEG for _ in range(5)]
            qrot = [P.tile([HD, TOK], bf16, name=f"qrot{u}", tag=f"qrot{u}")
                    for u in range(NUNITS)]
            v_sb = [P.tile([128, VTOT], bf16, name=f"vsb{m}", tag=f"vsb{m}")
                    for m in range(TOK // 128)]
            ctxn = [[P.tile([HD, SEGLEN], bf16, name=f"ctxn{j}_{s}",
                            tag=f"ctxn{j}_{s}")
                     for s in range(NSEG)] for j in range(HPC)]

            # PSUM budget (8 banks): qk-proj, pi-swap and out-proj share one
            # 3-slot pool (same shape/tag, mostly disjoint); v chunks and
            # softmax denominators share 1 slot (disjoint in time);
            # scores 2; ctx 2.
            with tc.tile_pool(name="ps_a", bufs=3, space="PSUM") as PSA, \
                 tc.tile_pool(name="ps_v", bufs=1, space="PSUM") as PSV, \
                 tc.tile_pool(name="ps_st", bufs=2, space="PSUM") as PST, \
                 tc.tile_pool(name="ps_ctx", bufs=2, space="PSUM") as PSC, \
                 tc.tile_pool(name="work", bufs=3) as W, \
                 tc.tile_pool(name="workd", bufs=6) as WD:

                ones_sb = P.tile([128, 1], bf16, name="ones_sb", tag="ones")
                nc.vector.memset(ones_sb[:], 1.0)

                est = {}    # (s, j) -> list of 4 exp'd score tiles

                def qkproj(t, s):
                    sc = slice(SEGLEN * s, SEGLEN * (s + 1))
                    qk_ps = PSA.tile([128, SEGLEN], f32, name=f"qkps{t}_{s}",
                                     tag="mm512")
                    for e in range(KCH):
                        nc.tensor.matmul(qk_ps[:],
                                         wqk_sb[e][:, 128 * t:128 * (t + 1)],
                                         xt_sb[e][:, sc],
                                         start=(e == 0), stop=(e == KCH - 1))
                    q_sb = W.tile([128, SEGLEN], bf16, name=f"qsb{t}_{s}",
                                  tag="qsb", bufs=10)
                    nc.scalar.activation(q_sb[:], qk_ps[:], AF.Identity,
                                         bias=bias_sb[:, t:t + 1])
                    qkp_sb[t][s] = q_sb

                def vchunk(m):
                    mc = slice(128 * m, 128 * (m + 1))
                    v_ps = PSV.tile([128, VTOT], f32, name=f"vps{m}", tag="vps")
                    for e in range(KCH):
                        nc.tensor.matmul(v_ps[:], xt_sb[e][:, mc], wv_sb[e][:],
                                         start=(e == 0), stop=(e == KCH - 1))
                    # v_sb = v_ps + (bias | ones) row pattern
                    nc.vector.scalar_tensor_tensor(v_sb[m][:], v_ps[:], 1.0,
                                                   vpat_sb[:], ALU.mult,
                                                   ALU.add)

                def rotary(tr, s):
                    sc = slice(SEGLEN * s, SEGLEN * (s + 1))
                    qsw_ps = PSA.tile([128, SEGLEN], f32, name=f"qsw{tr}_{s}",
                                      tag="mm512")
                    srcs = PI_BLOCKS[tr]
                    for i, tp in enumerate(srcs):
                        nc.tensor.matmul(qsw_ps[:],
                                         pit_sb[tp][:, 128 * tr:128 * (tr + 1)],
                                         qkp_sb[tp][s][:],
                                         start=(i == 0),
                                         stop=(i == len(srcs) - 1))
                    t1 = W.tile([128, SEGLEN], f32, name=f"t1_{tr}_{s}",
                                tag="t1", bufs=4)
                    nc.vector.tensor_tensor(t1[:], qkp_sb[tr][s][:],
                                            cos_sb[tr][:, sc], ALU.mult)
                    t2 = W.tile([128, SEGLEN], f32, name=f"t2_{tr}_{s}",
                                tag="t2", bufs=4)
                    nc.vector.tensor_tensor(t2[:], qsw_ps[:],
                                            sin_sb[tr][:, sc], ALU.mult)
                    rp = W.tile([128, SEGLEN], bf16, name=f"rotp{tr}_{s}",
                                tag="rotp", bufs=6)
                    nc.vector.tensor_tensor(rp[:], t1[:], t2[:], ALU.add)
                    for (u, po, toff, ln) in UNPACK_PIECES[tr]:
                        nc.scalar.dma_start(qrot[u][po:po + ln, sc],
                                            rp[toff:toff + ln, :])

                def scores(s, j):
                    sc = slice(SEGLEN * s, SEGLEN * (s + 1))
                    lst = []
                    for tkc in range(SEGLEN // 128):
                        kc = slice(SEGLEN * s + 128 * tkc,
                                   SEGLEN * s + 128 * (tkc + 1))
                        st_ps = PST.tile([128, SEGLEN], f32,
                                         name=f"st{j}_{s}_{tkc}", tag="stps")
                        nc.tensor.matmul(st_ps[:], qrot[2 * j + 1][:, kc],
                                         qrot[2 * j][:, sc],
                                         start=True, stop=True)
                        e_sb = WD.tile([128, SEGLEN], bf16,
                                       name=f"est{j}_{s}_{tkc}", tag="est",
                                       bufs=8)
                        nc.scalar.activation(e_sb[:], st_ps[:], AF.Exp)
                        lst.append(e_sb)
                    est[(s, j)] = lst

                def ctx(s, j):
                    lst = est.pop((s, j))
                    ctx_ps = PSC.tile([HD, SEGLEN], f32, name=f"ctxps{j}_{s}",
                                      tag="ctxps")
                    den_ps = PSV.tile([1, SEGLEN], f32, name=f"denps{j}_{s}",
                                      tag="vps")
                    for tkc in range(SEGLEN // 128):
                        nc.tensor.matmul(den_ps[:], ones_sb[:], lst[tkc][:],
                                         start=(tkc == 0), stop=(tkc == 3))
                    for tkc in range(SEGLEN // 128):
                        nc.tensor.matmul(ctx_ps[:],
                                         v_sb[4 * s + tkc][:, VW * j:VW * (j + 1)],
                                         lst[tkc][:],
                                         start=(tkc == 0), stop=(tkc == 3))
                    rec = WD.tile([1, SEGLEN], f32, name=f"rec{j}_{s}",
                                  tag="rec", bufs=3)
                    nc.vector.reciprocal_approx_fast(rec[:], den_ps[:])
                    bc = WD.tile([HD, SEGLEN], f32, name=f"bc{j}_{s}",
                                 tag="bc", bufs=3)
                    nc.gpsimd.partition_broadcast(bc[:], rec[:])
                    nc.vector.tensor_tensor(ctxn[j][s][:], ctx_ps[:, :],
                                            bc[:], ALU.mult)

                def oproj(e, s):
                    o_ps = PSA.tile([128, SEGLEN], f32, name=f"ops{e}_{s}",
                                    tag="mm512")
                    for j in range(HPC):
                        nc.tensor.matmul(o_ps[:],
                                         wp_sb[j][:, 128 * e:128 * (e + 1)],
                                         ctxn[j][s][:],
                                         start=(j == 0), stop=(j == HPC - 1))
                    o_sb = W.tile([128, SEGLEN], f16, name=f"osb{e}_{s}",
                                  tag="osb", bufs=4)
                    if e % 2 == 0:
                        nc.vector.tensor_copy(o_sb[:], o_ps[:])
                    else:
                        nc.scalar.activation(o_sb[:], o_ps[:], AF.Identity)
                    nc.sync.dma_start(
                        out_d[128 * e:128 * (e + 1),
                              SEGLEN * s:SEGLEN * (s + 1)], o_sb[:])

                # ---- phase B: packed qk projection + v + rotary, pipelined;
                # attention for head j gates only on packed tiles <= j+1 ----
                VCH = {0: [0, 1, 2], 1: [3, 4, 5], 2: [6, 7], 3: [], 4: []}
                for t in range(5):
                    for s in range(NSEG):
                        qkproj(t, s)
                    for m in VCH[t]:
                        vchunk(m)
                    ready = [t - 1] if t < 4 else [3, 4]
                    for tr in ready:
                        if tr < 0:
                            continue
                        for s in range(NSEG):
                            rotary(tr, s)
                    # head j's q/k live in packed tiles t=ceil? j0:t0,t1 ...
                    # rotary(tr) done for tr<=t-1 (t<4). scores (s, j) needs
                    # rotary of tiles up to j+1.
                    if t >= 2:
                        j = t - 2
                        for s in range(NSEG):
                            scores(s, j)
                            ctx(s, j)

                # remaining heads (j=3 needs rotary t4) -> s-major with
                # out-proj of seg 0 interleaved to keep PE dense
                scores(0, 3)
                ctx(0, 3)
                oproj(0, 0)
                oproj(1, 0)
                scores(1, 3)
                oproj(2, 0)
                oproj(3, 0)
                ctx(1, 3)
                for e in range(4, KCH):
                    oproj(e, 0)
                for e in range(KCH):
                    oproj(e, 1)

    nc.compile()
    return nc


def _prep_inputs(x, rotary_pos_emb, qkv_w, qkv_b):
    """Build per-core input shards (host-side layout/constant prep)."""
    x2 = np.asarray(x, np.float32).reshape(SEQ, EMBED)
    rope = np.asarray(rotary_pos_emb, np.float32)
    qkv_w = np.asarray(qkv_w, np.float32)
    qkv_b = np.asarray(qkv_b, np.float32)

    # packed rotary multipliers: packed row p = 80u + d -> r = d % 40
    r_idx = np.tile(np.arange(HD) % RH, NUNITS)      # [640]
    cos_full = np.cos(rope)[:, r_idx].T.astype(BF)   # [640, 2048]
    sin_full = np.sin(rope)[:, r_idx].T.astype(BF)

    # packed swap permutation (sign folded), block-diagonal per 80-row unit:
    # within a unit, row d<40 reads -(d+40), row d>=40 reads +(d-40)
    D = NUNITS * HD
    Pi = np.zeros((D, D), np.float32)
    for u in range(NUNITS):
        o = HD * u
        for i in range(RH):
            Pi[o + i, o + i + RH] = -1.0
            Pi[o + i + RH, o + i] = 1.0
    pit = np.ascontiguousarray(Pi.T).astype(BF)

    in_maps = []
    for c in range(N_CORES):
        sg, hg = divmod(c, HPC)
        toks = slice(TOK * sg, TOK * (sg + 1))
        heads = [HPC * hg + j for j in range(HPC)]

        xa = np.ascontiguousarray(x2[toks].T)

        # interleaved packing: unit 2j = q of head j, unit 2j+1 = k
        wqk = np.empty((EMBED, NUNITS * HD), np.float32)
        bias_flat = np.empty(NUNITS * HD, np.float32)
        for j, h in enumerate(heads):
            oq, ok = HD * 2 * j, HD * (2 * j + 1)
            wqk[:, oq:oq + HD] = qkv_w[HD * h:HD * (h + 1), :].T * SCALE
            bias_flat[oq:oq + HD] = qkv_b[HD * h:HD * (h + 1)] * SCALE
            ko = EMBED + HD * h
            wqk[:, ok:ok + HD] = qkv_w[ko:ko + HD, :].T
            bias_flat[ok:ok + HD] = qkv_b[ko:ko + HD]
        bias = np.ascontiguousarray(bias_flat.reshape(5, 128).T)

        # v weights: per head block of 80 cols; eviction adds the v bias row
        wv = np.zeros((EMBED, VTOT), np.float32)
        vpat_row = np.zeros(VTOT, np.float32)
        for j, h in enumerate(heads):
            vo = 2 * EMBED + HD * h
            wv[:, VW * j:VW * j + HD] = qkv_w[vo:vo + HD, :].T
            vpat_row[VW * j:VW * j + HD] = qkv_b[vo:vo + HD]
        vpat = np.ascontiguousarray(np.broadcast_to(vpat_row, (128, VTOT)))

        wp = np.empty((HPC * HD, EMBED), np.float32)
        for j, h in enumerate(heads):
            wp[HD * j:HD * (j + 1), :] = _PROJ_W[:, HD * h:HD * (h + 1)].T

        in_maps.append({
            "xt": xa.astype(BF),
            "wqk": wqk.astype(BF),
            "wv": wv.astype(BF),
            "vpat": vpat.astype(BF),
            "wp": wp.astype(BF),
            "biasqk": bias,
            "cosm": np.ascontiguousarray(cos_full[:, toks]),
            "sinm": np.ascontiguousarray(sin_full[:, toks]),
            "pit": pit,
        })
    return in_maps


_PROJ_W = None


def run_on_device(inputs, trace=False, trace_cores=None):
    """Shard, run on 8 NeuronCores, gather. Returns (output, BassKernelResults)."""
    global _PROJ_W
    from concourse import bass_utils

    x = np.asarray(inputs["x"], np.float32)
    cu = np.asarray(inputs["cu_seqlens"]).tolist()
    assert cu == [0, 512, 1024, 1536, 2048], (
        f"kernel compiled for 4x512 segments, got cu_seqlens={cu}")
    assert x.shape == (SEQ, 1, EMBED)

    _PROJ_W = np.asarray(inputs["proj_w"], np.float32)
    in_maps = _prep_inputs(x, inputs["rotary_pos_emb"],
                           inputs["qkv_w"], inputs["qkv_b"])

    if "nc" not in _CACHE:
        _CACHE["nc"] = _build_program()
    nc = _CACHE["nc"]

    kw = {}
    if trace:
        kw = dict(trace=True, trace_cores=trace_cores or [0])
    res = bass_utils.run_bass_kernel_spmd(nc, in_maps,
                                          core_ids=list(range(N_CORES)), **kw)

    proj_b = np.asarray(inputs["proj_b"], np.float32)
    out = np.empty((SEQ, EMBED), np.float32)
    for sg in range(2):
        acc = res.results[HPC * sg + 0]["outT"].astype(np.float32)
        for hg in range(1, HPC):
            acc = acc + res.results[HPC * sg + hg]["outT"].astype(np.float32)
        out[TOK * sg:TOK * (sg + 1)] = acc.T
    out += proj_b
    return out.reshape(SEQ, 1, EMBED), res


def kernel(**inputs):
    out, _ = run_on_device(inputs, trace=False)
    return out
